# revision 23
# baseline (speedup 1.0000x reference)
"""Trainium2 Bass kernel for nn_Block_74380243632568 (attention + MoE block).

Strategy: token-parallel SPMD over 8 NeuronCores, zero collectives.
 - Core c owns 4 query tiles of 128 tokens, zig-zag assigned for causal
   balance: global token tiles [c, 15-c, 16+c, 31-c].
 - K/V are computed for all 4096 tokens on every core (replicated), so no
   cross-core communication is needed; the host assembles output shards.
 - Causal attention runs a per-core-uniform 34-slot schedule (4 diagonal +
   30 full 128x128 tiles); kv/q addressing is driven by per-core int32
   tables loaded into engine registers, keeping the program SPMD-identical.
 - Softmax runs without max-subtraction (scores are O(1) here); the
 	denominator is carried as an extra ones-column in the V operand.
 - MoE: top-2 routing on device via max/is_equal; all 8 experts + the
   shared expert run densely on the core's own 512 tokens with exact
   per-token combine weights (non-selected experts weigh 0).
 - Matmuls run in float32r (fp32 storage, ~1e-4 relative rounding).
 - aux-loss partial sums (top-1 counts, prob sums) are per-core outputs,
   combined on the host.
"""

import numpy as np

import concourse.bass as bass
import concourse.mybir as mybir
import concourse.tile as tile
from concourse import bacc
from concourse.bass import ds, ts
from concourse.bass_utils import run_bass_kernel_spmd
from concourse.masks import make_identity

P = 128
B, S, D = 2, 2048, 1024
H, HKV, HD = 16, 4, 64
E = 8
HID = 684
NTOK = B * S           # 4096
NT = NTOK // P         # 32 token tiles
TPB = S // P           # 16 tiles per batch
KT = D // P            # 8 contraction tiles over D
OWN = 4                # own q tiles per core
NSLOT = 34             # 4 diag + 30 full
HIDT = 6               # ceil(684/128)
HID_LAST = HID - 5 * P  # 44
NEXP = 9               # 8 experts + shared
F32 = mybir.dt.float32
F32R = mybir.dt.float32r
I32 = mybir.dt.int32
ALU = mybir.AluOpType
ACTF = mybir.ActivationFunctionType

_CACHED = {}


def build_kernel():
    AX = mybir.AxisListType.X
    nc = bacc.Bacc("TRN2", target_bir_lowering=False, debug=False, num_devices=8)
    PE = mybir.EngineType.PE
    DVE = mybir.EngineType.DVE

    def param(name, shape, dt=F32R):
        return nc.declare_dram_parameter(name, shape, dt, isOutput=False)

    xT_d = param("xT", [D, NTOK])
    xTown_d = param("xT_own", [D, OWN * P])
    x_own_d = param("x_own", [OWN * P, D], F32)
    x_all_d = param("x_all", [NTOK, D], F32)
    wq_d = param("wq", [D, D])
    wkv_d = param("wkv", [D, 512])
    wo_d = param("wo", [D, D])
    rw_d = param("rw", [D, E])
    w1_d = param("w1", [NEXP, D, HID])
    w3_d = param("w3", [NEXP, D, HID])
    w2_d = param("w2", [NEXP, HID, D])
    cos_all_d = param("cos_all", [S, HD // 2], F32)
    sin_all_d = param("sin_all", [S, HD // 2], F32)
    cos_own_d = param("cos_own", [P, OWN, HD // 2], F32)
    sin_own_d = param("sin_own", [P, OWN, HD // 2], F32)
    # per-slot table: [kv_col, q128, q512, pad, voff_kvh0..3] x NSLOT
    slots_d = param("slots", [1, NSLOT * 8], I32)
    own_d = param("own", [1, 8], I32)  # [pad, pad, tidx0..3, pad, pad]

    y_d = nc.declare_dram_parameter("y", [OWN * P, D], F32, isOutput=True)
    aux_d = nc.declare_dram_parameter("aux_parts", [2, E], F32, isOutput=True)

    from contextlib import ExitStack

    with tile.TileContext(nc) as tc, ExitStack() as stk:
        const = stk.enter_context(tc.tile_pool(name="const", bufs=1))
        persist = stk.enter_context(tc.tile_pool(name="persist", bufs=1))

        ident = const.tile([P, P], F32)
        make_identity(nc, ident)
        ident_r = const.tile([P, P], F32R)
        nc.vector.tensor_copy(out=ident_r, in_=ident)
        ones_col = const.tile([P, 1], F32R)
        nc.vector.memset(ones_col.bitcast(F32), 1.0)
        ones_row = const.tile([P, P], F32R)
        nc.vector.memset(ones_row.bitcast(F32), 1.0)
        eps_t = const.tile([P, 1], F32)
        nc.vector.memset(eps_t, 1e-5)

        cos_own = const.tile([P, OWN, HD // 2], F32)
        nc.sync.dma_start(out=cos_own, in_=cos_own_d[:, :, :])
        sin_own = const.tile([P, OWN, HD // 2], F32)
        nc.sync.dma_start(out=sin_own, in_=sin_own_d[:, :, :])
        cos_all = const.tile([P, TPB, HD // 2], F32)
        nc.sync.dma_start(out=cos_all, in_=cos_all_d.rearrange("(t p) f -> p t f", p=P))
        sin_all = const.tile([P, TPB, HD // 2], F32)
        nc.sync.dma_start(out=sin_all, in_=sin_all_d.rearrange("(t p) f -> p t f", p=P))

        slots_sb = const.tile([1, NSLOT * 8], I32)
        nc.sync.dma_start(out=slots_sb, in_=slots_d[:, :])
        own_sb = const.tile([1, 8], I32)
        nc.sync.dma_start(out=own_sb, in_=own_d[:, :])

        def loadval(ap, lo, hi, engines=(PE,)):
            _, vals = nc.values_load_multi_w_load_instructions(
                ap,
                engines=list(engines),
                min_val=lo,
                max_val=hi,
                skip_runtime_bounds_check=True,
            )
            return vals

        own_tidx = loadval(own_sb[0:1, 2:6], 0, NT - 1, engines=(DVE,))

        # persistent across all phases
        rstd_all = persist.tile([P, NT], F32)
        h_sb = persist.tile([P, OWN, D], F32)
        cw_sb = persist.tile([P, OWN, NEXP], F32)
        acc_cnt = persist.tile([P, E], F32R)
        acc_sp = persist.tile([P, E], F32R)

        nc.vector.memset(cw_sb, 1.0)
        nc.vector.memset(acc_cnt.bitcast(F32), 0.0)
        nc.vector.memset(acc_sp.bitcast(F32), 0.0)

        # ---------------- phase 0: rms stats for all tokens ----------------
        with (
            tc.tile_pool(name="xin", bufs=3) as xin,
            tc.tile_pool(name="stat", bufs=4) as stat,
        ):
            for tt in range(NT):
                x_t = xin.tile([P, D], F32, tag="xt")
                nc.sync.dma_start(out=x_t, in_=x_all_d[ts(tt, P), :])
                st = stat.tile([P, 2, 6], F32, tag="st")
                for sg in range(2):
                    nc.vector.bn_stats(out=st[:, sg, :], in_=x_t[:, ts(sg, 512)])
                mv = stat.tile([P, 2], F32, tag="mv")
                nc.vector.bn_aggr(out=mv, in_=st)
                msq = stat.tile([P, 1], F32, tag="msq")
                nc.vector.tensor_tensor(out=msq, in0=mv[:, 0:1], in1=mv[:, 0:1], op=ALU.mult)
                nc.vector.tensor_tensor(out=msq, in0=msq, in1=mv[:, 1:2], op=ALU.add)
                sq = stat.tile([P, 1], F32, tag="sq")
                nc.scalar.activation(out=sq, in_=msq, func=ACTF.Sqrt, bias=eps_t, scale=1.0)
                nc.vector.reciprocal(out=rstd_all[:, tt : tt + 1], in_=sq)

        # attention-scoped persistents (freed before the MoE phase)
        with tc.tile_pool(name="attnbig", bufs=1) as attnbig:
            kT = attnbig.tile([P, 2, NTOK], F32R)
            vhat = attnbig.tile([P, NT, HKV, HD + 1], F32R)
            qTz = attnbig.tile([P, H, OWN * P], F32R)
            outT = attnbig.tile([P, KT, OWN * P], F32R)
            nc.vector.memset(vhat.bitcast(F32), 1.0)
            nc.vector.memset(qTz.bitcast(F32), 0.0)

            # ------------- phase 1a: K/V projections + rope + transpose ------
            with (
                tc.tile_pool(name="xTp", bufs=1) as xTp,
                tc.tile_pool(name="wkvp", bufs=1) as wkvp,
                tc.tile_pool(name="kvps", bufs=2, space="PSUM") as kvps,
                tc.tile_pool(name="trps", bufs=2, space="PSUM") as trps,
                tc.tile_pool(name="ropet", bufs=2) as rope_tmp,
                tc.tile_pool(name="kvtmp", bufs=2) as kvtmp,
            ):
                def rope(dst, src, cos_ap, sin_ap, nh):
                    se = src[:, :, 0::2]
                    so = src[:, :, 1::2]
                    cb = cos_ap[:, None, :].to_broadcast([P, nh, HD // 2])
                    sb_ = sin_ap[:, None, :].to_broadcast([P, nh, HD // 2])
                    t1 = rope_tmp.tile([P, nh, HD // 2], F32, tag=f"rt1_{nh}")
                    t2 = rope_tmp.tile([P, nh, HD // 2], F32, tag=f"rt2_{nh}")
                    nc.vector.tensor_tensor(out=t1, in0=se, in1=cb, op=ALU.mult)
                    nc.vector.tensor_tensor(out=t2, in0=so, in1=sb_, op=ALU.mult)
                    nc.vector.tensor_tensor(out=dst[:, :, 0::2], in0=t1, in1=t2, op=ALU.subtract)
                    nc.vector.tensor_tensor(out=t1, in0=se, in1=sb_, op=ALU.mult)
                    nc.vector.tensor_tensor(out=t2, in0=so, in1=cb, op=ALU.mult)
                    nc.vector.tensor_tensor(out=dst[:, :, 1::2], in0=t1, in1=t2, op=ALU.add)

                wkv_sb = wkvp.tile([P, KT, 512], F32R)
                nc.sync.dma_start(out=wkv_sb, in_=wkv_d.rearrange("(k p) f -> p k f", p=P))
                for eighth in range(8):
                    xT_sb = xTp.tile([P, KT, NTOK // 8], F32R, tag="xtq")
                    nc.sync.dma_start(
                        out=xT_sb,
                        in_=xT_d.rearrange("(k p) n -> p k n", p=P)[
                            :, :, ts(eighth, NTOK // 8)
                        ],
                    )
                    for t in range(4):
                        g = eighth * 4 + t
                        ps = kvps.tile([P, 512], F32, tag="kvps")
                        for kt in range(KT):
                            nc.tensor.matmul(
                                ps,
                                lhsT=xT_sb[:, kt, ts(t, P)],
                                rhs=wkv_sb[:, kt, :],
                                start=(kt == 0),
                                stop=(kt == KT - 1),
                            )
                        k_tm = kvtmp.tile([P, HKV, HD], F32, tag="ktm")
                        nc.vector.tensor_scalar(
                            out=k_tm,
                            in0=ps[:, 0:256].rearrange("p (h d) -> p h d", d=HD),
                            scalar1=rstd_all[:, g : g + 1],
                            scalar2=None,
                            op0=ALU.mult,
                        )
                        nc.vector.tensor_scalar(
                            out=vhat[:, g, :, 0:HD],
                            in0=ps[:, 256:512].rearrange("p (h d) -> p h d", d=HD),
                            scalar1=rstd_all[:, g : g + 1],
                            scalar2=None,
                            op0=ALU.mult,
                        )
                        kr = kvtmp.tile([P, HKV, HD], F32, tag="krop")
                        rope(kr, k_tm, cos_all[:, g % TPB, :], sin_all[:, g % TPB, :], HKV)
                        krf = kr.rearrange("p h d -> p (h d)")
                        for ft in range(2):
                            tp = trps.tile([P, P], F32, tag="trp")
                            nc.tensor.transpose(tp, krf[:, ts(ft, P)], ident)
                            nc.vector.tensor_copy(out=kT[:, ft, ts(g, P)], in_=tp)

            # ------------- phase 1b: Q projection + rope + transpose ---------
            with (
                tc.tile_pool(name="xqp", bufs=1) as xqp,
                tc.tile_pool(name="wqp", bufs=2) as wqp,
                tc.tile_pool(name="qps", bufs=2, space="PSUM") as qps,
                tc.tile_pool(name="qtrps", bufs=2, space="PSUM") as qtrps,
                tc.tile_pool(name="qropet", bufs=2) as qrope_tmp,
                tc.tile_pool(name="qtmp", bufs=2) as qtmp,
            ):
                xq_sb = xqp.tile([P, KT, OWN * P], F32R)
                nc.sync.dma_start(out=xq_sb, in_=xTown_d.rearrange("(k p) n -> p k n", p=P))
                for qt in range(OWN):
                    q_tm = qtmp.tile([P, H, HD], F32, tag="qtm")
                    q_tm_f = q_tm.rearrange("p h d -> p (h d)")
                    for fh in range(2):
                        ps = qps.tile([P, 512], F32, tag="qps")
                        for kt in range(KT):
                            wq_t = wqp.tile([P, 512], F32R, tag="wqt")
                            nc.sync.dma_start(out=wq_t, in_=wq_d[ts(kt, P), ts(fh, 512)])
                            nc.tensor.matmul(
                                ps,
                                lhsT=xq_sb[:, kt, ts(qt, P)],
                                rhs=wq_t,
                                start=(kt == 0),
                                stop=(kt == KT - 1),
                            )
                        nc.vector.tensor_scalar(
                            out=q_tm_f[:, ts(fh, 512)],
                            in0=ps,
                            scalar1=rstd_all[:, ds(own_tidx[qt], 1)],
                            scalar2=None,
                            op0=ALU.mult,
                        )
                    se = q_tm[:, :, 0::2]
                    so = q_tm[:, :, 1::2]
                    cb = cos_own[:, qt, None, :].to_broadcast([P, H, HD // 2])
                    sb_ = sin_own[:, qt, None, :].to_broadcast([P, H, HD // 2])
                    qr = qtmp.tile([P, H, HD], F32, tag="qrop")
                    t1 = qrope_tmp.tile([P, H, HD // 2], F32, tag="qt1")
                    t2 = qrope_tmp.tile([P, H, HD // 2], F32, tag="qt2")
                    nc.vector.tensor_tensor(out=t1, in0=se, in1=cb, op=ALU.mult)
                    nc.vector.tensor_tensor(out=t2, in0=so, in1=sb_, op=ALU.mult)
                    nc.vector.tensor_tensor(out=qr[:, :, 0::2], in0=t1, in1=t2, op=ALU.subtract)
                    nc.vector.tensor_tensor(out=t1, in0=se, in1=sb_, op=ALU.mult)
                    nc.vector.tensor_tensor(out=t2, in0=so, in1=cb, op=ALU.mult)
                    nc.vector.tensor_tensor(out=qr[:, :, 1::2], in0=t1, in1=t2, op=ALU.add)
                    for qa in [0, 1, 2, 3, 8, 9, 10, 11]:
                        qb = qa + 4
                        pr = qtmp.tile([P, P], F32, tag="pr")
                        nc.vector.tensor_copy(
                            out=pr.rearrange("p (h d) -> p h d", h=2),
                            in_=qr[:, qa : qb + 1 : 4, :],
                        )
                        tp = qtrps.tile([P, P], F32, tag="qtrp")
                        nc.tensor.transpose(tp, pr, ident)
                        nc.vector.tensor_copy(
                            out=qTz[0:HD, qa, ts(qt, P)], in_=tp[0:HD, :]
                        )
                        nc.vector.tensor_copy(
                            out=qTz[HD:P, qb, ts(qt, P)], in_=tp[HD:P, :]
                        )

            # ---------------- phase 2: attention slots ----------------
            vflat = vhat.rearrange("p t h f -> p (t h f)")
            with (
                tc.tile_pool(name="scps", bufs=2, space="PSUM") as scps,
                tc.tile_pool(name="avps", bufs=1, space="PSUM") as avps,
                tc.tile_pool(name="drps", bufs=1, space="PSUM") as drps,
                tc.tile_pool(name="etp", bufs=3) as etp,
                tc.tile_pool(name="dsbp", bufs=2) as dsbp,
                tc.tile_pool(name="finp", bufs=1) as finp,
            ):
                for kvh in range(HKV):
                    av = avps.tile([P, OWN * 512], F32, tag="av")
                    for s in range(NSLOT):
                        q128 = loadval(slots_sb[0:1, s * 8 + 1 : s * 8 + 2], 0, 384)[0]
                        q512 = loadval(slots_sb[0:1, s * 8 + 2 : s * 8 + 3], 0, 1536)[0]
                        voff = loadval(
                            slots_sb[0:1, s * 8 + 4 + kvh : s * 8 + 5 + kvh],
                            0,
                            NT * HKV * (HD + 1) - (HD + 1),
                            engines=(DVE,),
                        )[0]
                        kv_col_d = loadval(
                            slots_sb[0:1, s * 8 : s * 8 + 1], 0, NTOK - P, engines=(DVE,)
                        )[0]
                        kstage = etp.tile([P, P], F32R, tag="kstage")
                        nc.vector.tensor_copy(
                            out=kstage, in_=kT[:, kvh // 2, ds(kv_col_d, P)]
                        )
                        vstage = etp.tile([P, HD + 1], F32R, tag="vstage")
                        nc.vector.tensor_copy(out=vstage, in_=vflat[:, ds(voff, HD + 1)])
                        sc = scps.tile([P, 512], F32, tag="sc")
                        for ql in range(4):
                            qh = kvh * 4 + ql
                            nc.tensor.matmul(
                                sc[:, ts(ql, P)],
                                lhsT=kstage,
                                rhs=qTz[:, qh, ds(q128, P)],
                                start=True,
                                stop=True,
                            )
                        eT = etp.tile([P, 4, P], F32R, tag="et")
                        nc.scalar.activation(
                            out=eT.rearrange("p a b -> p (a b)"),
                            in_=sc,
                            func=ACTF.Exp,
                            scale=0.125,
                        )
                        if s < 4:
                            nc.gpsimd.affine_select(
                                out=eT,
                                in_=eT,
                                compare_op=ALU.is_ge,
                                fill=0.0,
                                base=0,
                                pattern=[[0, 4], [1, P]],
                                channel_multiplier=-1,
                            )
                        nc.tensor.matmul(
                            av[0 : HD + 1, ds(q512, 512)],
                            lhsT=vstage,
                            rhs=eT.rearrange("p a b -> p (a b)"),
                            start=(s < 4),
                            stop=False,
                            skip_group_check=True,
                        )
                    # finalize this kv head
                    for ql in range(4):
                        qh = kvh * 4 + ql
                        ft = qh // 2
                        row = (qh % 2) * HD
                        dsb = dsbp.tile([P, 512], F32R, tag="dsb")
                        nc.vector.tensor_copy(
                            out=dsb[HD : HD + 1, :].rearrange("d (a b) -> d a b", b=P),
                            in_=av[HD : HD + 1, :].rearrange(
                                "d (qt ql p) -> d ql qt p", ql=4, p=P
                            )[:, ql, :, :],
                        )
                        dr_ps = drps.tile([P, 512], F32, tag="drp")
                        nc.tensor.matmul(
                            dr_ps,
                            lhsT=ones_row[HD : HD + 1, 0:P],
                            rhs=dsb[HD : HD + 1, :],
                            start=True,
                            stop=True,
                        )
                        dr_sb = finp.tile([P, 512], F32, tag="drsb")
                        nc.vector.tensor_copy(out=dr_sb, in_=dr_ps)
                        rdr = finp.tile([P, 512], F32, tag="rdr")
                        nc.vector.reciprocal(out=rdr, in_=dr_sb)
                        num = av[0:HD, :].rearrange("d (qt ql p) -> d ql qt p", ql=4, p=P)[
                            :, ql, :, :
                        ]
                        rdr_v = rdr[0:HD, :].rearrange("d (qt p) -> d qt p", p=P)
                        if row == 0:
                            nc.vector.tensor_tensor(
                                out=outT[0:HD, ft, :].rearrange("d (qt p) -> d qt p", p=P),
                                in0=num,
                                in1=rdr_v,
                                op=ALU.mult,
                            )
                        else:
                            div_sb = finp.tile([HD, 512], F32R, tag="divsb")
                            nc.vector.tensor_tensor(
                                out=div_sb.rearrange("d (qt p) -> d qt p", p=P),
                                in0=num,
                                in1=rdr_v,
                                op=ALU.mult,
                            )
                            nc.sync.dma_start(out=outT[HD:P, ft, :], in_=div_sb)

            # ---------------- phase 3: wo + residual ----------------
            with (
                tc.tile_pool(name="wop", bufs=3) as wop,
                tc.tile_pool(name="wops", bufs=2, space="PSUM") as wops,
                tc.tile_pool(name="xop", bufs=2) as xop,
            ):
                for qt in range(OWN):
                    x_own_t = xop.tile([P, D], F32, tag="xo")
                    nc.sync.dma_start(out=x_own_t, in_=x_own_d[ts(qt, P), :])
                    for fh in range(2):
                        ps = wops.tile([P, 512], F32, tag="wops")
                        for kt in range(KT):
                            wo_t = wop.tile([P, 512], F32R, tag="wot")
                            nc.sync.dma_start(out=wo_t, in_=wo_d[ts(kt, P), ts(fh, 512)])
                            nc.tensor.matmul(
                                ps,
                                lhsT=outT[:, kt, ts(qt, P)],
                                rhs=wo_t,
                                start=(kt == 0),
                                stop=(kt == KT - 1),
                            )
                        nc.vector.tensor_tensor(
                            out=h_sb[:, qt, ts(fh, 512)],
                            in0=ps,
                            in1=x_own_t[:, ts(fh, 512)],
                            op=ALU.add,
                        )

        # ---------- phase 4: rms2 + hnT + router/top2 + aux ----------
        latebig = stk.enter_context(tc.tile_pool(name="latebig", bufs=1))
        hnT = latebig.tile([P, KT, OWN * P], F32R)
        with (
            tc.tile_pool(name="st2", bufs=4) as st2,
            tc.tile_pool(name="hn2", bufs=2) as hn2,
            tc.tile_pool(name="tr2", bufs=2, space="PSUM") as tr2,
            tc.tile_pool(name="rwp", bufs=1) as rwp,
            tc.tile_pool(name="lgps", bufs=2, space="PSUM") as lgps,
            tc.tile_pool(name="top2", bufs=6) as top2,
        ):
            rw_sb = rwp.tile([P, KT, E], F32R)
            nc.sync.dma_start(out=rw_sb, in_=rw_d.rearrange("(k p) e -> p k e", p=P))
            for qt in range(OWN):
                st = st2.tile([P, 2, 6], F32, tag="st")
                for sg in range(2):
                    nc.vector.bn_stats(out=st[:, sg, :], in_=h_sb[:, qt, ts(sg, 512)])
                mv = st2.tile([P, 2], F32, tag="mv")
                nc.vector.bn_aggr(out=mv, in_=st)
                msq = st2.tile([P, 1], F32, tag="msq")
                nc.vector.tensor_tensor(out=msq, in0=mv[:, 0:1], in1=mv[:, 0:1], op=ALU.mult)
                nc.vector.tensor_tensor(out=msq, in0=msq, in1=mv[:, 1:2], op=ALU.add)
                sq = st2.tile([P, 1], F32, tag="sq")
                nc.scalar.activation(out=sq, in_=msq, func=ACTF.Sqrt, bias=eps_t, scale=1.0)
                rstd2 = st2.tile([P, 1], F32, tag="rstd2")
                nc.vector.reciprocal(out=rstd2, in_=sq)
                hn_t = hn2.tile([P, D], F32, tag="hn")
                nc.vector.tensor_scalar(
                    out=hn_t, in0=h_sb[:, qt, :], scalar1=rstd2, scalar2=None, op0=ALU.mult
                )
                for ft in range(KT):
                    tp = tr2.tile([P, P], F32, tag="tr2")
                    nc.tensor.transpose(tp, hn_t[:, ts(ft, P)], ident)
                    nc.vector.tensor_copy(out=hnT[:, ft, ts(qt, P)], in_=tp)
                lg = lgps.tile([P, E], F32, tag="lg")
                for kt in range(KT):
                    nc.tensor.matmul(
                        lg,
                        lhsT=hnT[:, kt, ts(qt, P)],
                        rhs=rw_sb[:, kt, :],
                        start=(kt == 0),
                        stop=(kt == KT - 1),
                    )
                logits = top2.tile([P, E], F32, tag="logits")
                nc.vector.tensor_copy(out=logits, in_=lg)
                m1 = top2.tile([P, 1], F32, tag="m1")
                nc.vector.reduce_max(out=m1, in_=logits, axis=AX)
                negm1 = top2.tile([P, 1], F32, tag="negm1")
                nc.vector.tensor_scalar(
                    out=negm1, in0=m1, scalar1=-1.0, scalar2=None, op0=ALU.mult
                )
                el = top2.tile([P, E], F32, tag="el")
                nc.scalar.activation(out=el, in_=logits, func=ACTF.Exp, bias=negm1, scale=1.0)
                z = top2.tile([P, 1], F32, tag="z")
                nc.vector.reduce_sum(out=z, in_=el, axis=AX)
                rz = top2.tile([P, 1], F32, tag="rz")
                nc.vector.reciprocal(out=rz, in_=z)
                probs = top2.tile([P, E], F32, tag="probs")
                nc.vector.tensor_scalar(
                    out=probs, in0=el, scalar1=rz, scalar2=None, op0=ALU.mult
                )
                eq1 = top2.tile([P, E], F32, tag="eq1")
                nc.vector.tensor_scalar(
                    out=eq1, in0=logits, scalar1=m1, scalar2=None, op0=ALU.is_equal
                )
                l2 = top2.tile([P, E], F32, tag="l2")
                nc.vector.tensor_scalar(
                    out=l2, in0=eq1, scalar1=-1e30, scalar2=None, op0=ALU.mult
                )
                nc.vector.tensor_tensor(out=l2, in0=l2, in1=logits, op=ALU.add)
                m2 = top2.tile([P, 1], F32, tag="m2")
                nc.vector.reduce_max(out=m2, in_=l2, axis=AX)
                eq2 = top2.tile([P, E], F32, tag="eq2")
                nc.vector.tensor_scalar(
                    out=eq2, in0=l2, scalar1=m2, scalar2=None, op0=ALU.is_equal
                )
                mask = top2.tile([P, E], F32, tag="mask")
                nc.vector.tensor_tensor(out=mask, in0=eq1, in1=eq2, op=ALU.add)
                nc.vector.tensor_tensor(
                    out=cw_sb[:, qt, 0:E], in0=probs, in1=mask, op=ALU.mult
                )
                nc.vector.tensor_tensor(out=acc_cnt, in0=acc_cnt, in1=eq1, op=ALU.add)
                nc.vector.tensor_tensor(out=acc_sp, in0=acc_sp, in1=probs, op=ALU.add)

        # ---------------- phase 5: experts (accumulate into h_sb) ------------
        with (
            tc.tile_pool(name="wts", bufs=3) as wts,
            tc.tile_pool(name="h1ps", bufs=2, space="PSUM") as h1ps,
            tc.tile_pool(name="h3ps", bufs=2, space="PSUM") as h3ps,
            tc.tile_pool(name="eops", bufs=2, space="PSUM") as eops,
            tc.tile_pool(name="h2tp", bufs=2) as h2tp,
            tc.tile_pool(name="h1sp", bufs=1) as h1sp,
            tc.tile_pool(name="eotmp", bufs=2) as eotmp,
        ):
            for e in range(NEXP):
                h1s_all = h1sp.tile([P, HIDT, OWN * P], F32, tag="h1s")
                w1_sb = wts.tile([P, KT, HID], F32R, tag="wts")
                nc.sync.dma_start(out=w1_sb, in_=w1_d[e].rearrange("(k p) f -> p k f", p=P))
                for mi in range(HIDT):
                    msz = P if mi < HIDT - 1 else HID_LAST
                    h1p = h1ps.tile([P, OWN * P], F32, tag="h1p")
                    for kt in range(KT):
                        nc.tensor.matmul(
                            h1p[0:msz, :],
                            lhsT=w1_sb[:, kt, mi * P : mi * P + msz],
                            rhs=hnT[:, kt, :],
                            start=(kt == 0),
                            stop=(kt == KT - 1),
                        )
                    sg = eotmp.tile([P, OWN * P], F32, tag="sg")
                    nc.scalar.activation(
                        out=sg[0:msz, :], in_=h1p[0:msz, :], func=ACTF.Sigmoid
                    )
                    nc.vector.tensor_tensor(
                        out=h1s_all[0:msz, mi, :],
                        in0=sg[0:msz, :],
                        in1=h1p[0:msz, :],
                        op=ALU.mult,
                    )
                w3_sb = wts.tile([P, KT, HID], F32R, tag="wts")
                nc.sync.dma_start(out=w3_sb, in_=w3_d[e].rearrange("(k p) f -> p k f", p=P))
                h2T = h2tp.tile([P, HIDT, OWN * P], F32R, tag="h2t")
                nc.vector.memset(h2T.bitcast(F32), 0.0)
                for mi in range(HIDT):
                    msz = P if mi < HIDT - 1 else HID_LAST
                    h3p = h3ps.tile([P, OWN * P], F32, tag="h3p")
                    for kt in range(KT):
                        nc.tensor.matmul(
                            h3p[0:msz, :],
                            lhsT=w3_sb[:, kt, mi * P : mi * P + msz],
                            rhs=hnT[:, kt, :],
                            start=(kt == 0),
                            stop=(kt == KT - 1),
                        )
                    nc.vector.tensor_tensor(
                        out=h2T[0:msz, mi, :],
                        in0=h1s_all[0:msz, mi, :],
                        in1=h3p[0:msz, :],
                        op=ALU.mult,
                    )
                w2_sb = wts.tile([P, HIDT, D], F32R, tag="wts")
                nc.vector.memset(w2_sb[:, HIDT - 1, :].bitcast(F32), 0.0)
                nc.sync.dma_start(
                    out=w2_sb[:, 0 : HIDT - 1, :],
                    in_=w2_d[e][0 : 5 * P, :].rearrange("(k p) f -> p k f", p=P),
                )
                nc.sync.dma_start(
                    out=w2_sb[0:HID_LAST, HIDT - 1, :], in_=w2_d[e][5 * P : HID, :]
                )
                for qt in range(OWN):
                    for fh in range(2):
                        eop = eops.tile([P, 512], F32, tag="eop")
                        for kt in range(HIDT):
                            nc.tensor.matmul(
                                eop,
                                lhsT=h2T[:, kt, ts(qt, P)],
                                rhs=w2_sb[:, kt, ts(fh, 512)],
                                start=(kt == 0),
                                stop=(kt == HIDT - 1),
                            )
                        tmp = eotmp.tile([P, 512], F32, tag="eot")
                        nc.vector.tensor_scalar(
                            out=tmp,
                            in0=eop,
                            scalar1=cw_sb[:, qt, e : e + 1],
                            scalar2=None,
                            op0=ALU.mult,
                        )
                        nc.vector.tensor_tensor(
                            out=h_sb[:, qt, ts(fh, 512)],
                            in0=h_sb[:, qt, ts(fh, 512)],
                            in1=tmp,
                            op=ALU.add,
                        )

        # ---------------- phase 6: outputs ----------------
        with (
            tc.tile_pool(name="yout", bufs=3) as yout,
            tc.tile_pool(name="auxps", bufs=2, space="PSUM") as auxps,
        ):
            for qt in range(OWN):
                nc.sync.dma_start(out=y_d[ts(qt, P), :], in_=h_sb[:, qt, :])
            for i, acc in enumerate((acc_cnt, acc_sp)):
                ps = auxps.tile([1, E], F32, tag="aux")
                nc.tensor.matmul(ps, lhsT=ones_col, rhs=acc, start=True, stop=True)
                row = yout.tile([1, E], F32, tag="auxrow")
                nc.vector.tensor_copy(out=row, in_=ps)
                nc.sync.dma_start(out=aux_d[i : i + 1, :], in_=row)

    nc.compile()
    return nc


def host_prep(inputs):
    x = np.ascontiguousarray(np.asarray(inputs["x"], np.float32)).reshape(NTOK, D)
    xT = np.ascontiguousarray(x.T)
    g_attn = np.asarray(inputs["g_attn"], np.float32)
    g_ffn = np.asarray(inputs["g_ffn"], np.float32)
    wq = g_attn[:, None] * np.asarray(inputs["wq"], np.float32)
    wkv = g_attn[:, None] * np.concatenate(
        [np.asarray(inputs["wk"], np.float32), np.asarray(inputs["wv"], np.float32)], 1
    )
    wo = np.ascontiguousarray(np.asarray(inputs["wo"], np.float32))
    rw = g_ffn[:, None] * np.asarray(inputs["router_w"], np.float32)
    w1 = np.concatenate(
        [np.asarray(inputs["ew1"], np.float32), np.asarray(inputs["sw1"], np.float32)], 0
    ) * g_ffn[None, :, None]
    w3 = np.concatenate(
        [np.asarray(inputs["ew3"], np.float32), np.asarray(inputs["sw3"], np.float32)], 0
    ) * g_ffn[None, :, None]
    w2 = np.concatenate(
        [np.asarray(inputs["ew2"], np.float32), np.asarray(inputs["sw2"], np.float32)], 0
    )
    cos_t = np.ascontiguousarray(np.asarray(inputs["freqs_cos"], np.float32))
    sin_t = np.ascontiguousarray(np.asarray(inputs["freqs_sin"], np.float32))

    common = dict(
        xT=xT, x_all=x, wq=np.ascontiguousarray(wq), wkv=np.ascontiguousarray(wkv),
        wo=wo, rw=np.ascontiguousarray(rw), w1=np.ascontiguousarray(w1),
        w3=np.ascontiguousarray(w3), w2=np.ascontiguousarray(w2),
        cos_all=cos_t, sin_all=sin_t,
    )

    per_core, own_tiles_all = [], []
    for c in range(8):
        own = [c, 15 - c, 16 + c, 31 - c]
        own_tiles_all.append(own)
        own_rows = np.concatenate([np.arange(g * P, (g + 1) * P) for g in own])
        x_own = np.ascontiguousarray(x[own_rows])
        xT_own = np.ascontiguousarray(xT[:, own_rows])
        pos_t = [g % TPB for g in own]
        cos_own = np.stack([cos_t[pt * P : (pt + 1) * P] for pt in pos_t], 1)
        sin_own = np.stack([sin_t[pt * P : (pt + 1) * P] for pt in pos_t], 1)
        slots = [(qt, own[qt]) for qt in range(OWN)]
        for qt in range(OWN):
            g = own[qt]
            slots += [(qt, kv) for kv in range(TPB * (g // TPB), g)]
        assert len(slots) == NSLOT
        tab = np.zeros((NSLOT, 8), np.int32)
        for si, (qt, kv) in enumerate(slots):
            tab[si, 0] = kv * P
            tab[si, 1] = qt * P
            tab[si, 2] = qt * 512
            for kvh in range(HKV):
                tab[si, 4 + kvh] = kv * HKV * (HD + 1) + kvh * (HD + 1)
        ownv = np.zeros((1, 8), np.int32)
        ownv[0, 0] = c * P
        ownv[0, 1] = (15 - c) * P
        ownv[0, 2:6] = own
        per_core.append(
            dict(
                common,
                x_own=x_own,
                xT_own=xT_own,
                cos_own=np.ascontiguousarray(cos_own),
                sin_own=np.ascontiguousarray(sin_own),
                slots=tab.reshape(1, -1),
                own=ownv,
            )
        )
    return per_core, own_tiles_all


def assemble(results, own_tiles_all):
    y = np.zeros((NTOK, D), np.float32)
    cnt = np.zeros(E, np.float64)
    sp = np.zeros(E, np.float64)
    for c in range(8):
        y_own = results[c]["y"]
        for qt, g in enumerate(own_tiles_all[c]):
            y[g * P : (g + 1) * P] = y_own[qt * P : (qt + 1) * P]
        cnt += results[c]["aux_parts"][0]
        sp += results[c]["aux_parts"][1]
    density = cnt / NTOK
    meanp = sp / NTOK
    aux = np.float32(0.01 * float((density * meanp).sum()) * E)
    return y.reshape(B, S, D), aux


def kernel(**inputs):
    if "nc" not in _CACHED:
        _CACHED["nc"] = build_kernel()
    nc = _CACHED["nc"]
    per_core, own_tiles_all = host_prep(inputs)
    res = run_bass_kernel_spmd(nc, per_core, list(range(8)))
    return assemble(res.results, own_tiles_all)
